# revision 10
# baseline (speedup 1.0000x reference)
"""Trainium2 Bass kernel for SAGAN-style self-attention.

Reference computation (per sample, B=8 samples over 8 cores):
    xf = x.reshape(N=4096, C=64)
    f = xf @ Wf + bf            # [N, 8]
    g = xf @ Wg + bg            # [N, 8]
    h = xf @ Wh + bh            # [N, 64]
    s = g @ f.T                 # [N, N]
    beta = softmax(s, axis=-1)
    out = gamma * (beta @ h) + xf

Device-side layout (per core, sample i) -- fp8 DoubleRow design:
  - st = s.T computed as [m(part), n(free)] tiles, 2-chunk spans (one
    PSUM 2-bank slot each, 3 rotating slots in banks 0-5).
  - The Schraudolph affine A8*s + B8 and the per-softmax-row shift both
    ride the st matmul: f/g projection weights are scaled by sqrt(A8)
    on the host, and a 9th contraction row (ones on the f side, bf16
    -(A8*shift_n - B8) on the g side, shift_n = exact host row max
    minus a 4.5 margin) is appended.  PSUM st is then directly in
    "fp8e4m3 byte" units.
  - exp of the 16.7M logits is split between ScalarE (true exp via
    activation(scale=1/A8, bias=-B8/A8), fp8e4 output; the fp32->fp8
    converter flushes small values to zero) and VectorE (a single
    max(st,0) ALU op written as uint8 IS the e4m3 Schraudolph byte;
    clamping at 0 makes wrap-around impossible and the upper bound
    byte<=~111 < 120-NaN-zone is guaranteed by the host shift margin).
  - the [N,N] softmax normalizer rides the beta@h matmul as a 65th
    "ones" column of h (h stored fp8e4 in 80-byte slots; gamma folded
    into Wh/bh on the host); per-row shift cancels in the o/Z ratio.
  - beta@h runs in fp8 DoubleRow mode: one matmul per 2-chunk span
    (lhsT = h pair [128,2,65], rhs = exp tile [128,2,512]) accumulates
    a whole S-block into a single PSUM bank (banks 6/7 alternate).
    DoubleRow does two 128-deep fp8 contractions per streamed column,
    halving the o-matmul stream vs bf16.
  - PE program emits spans in PAIRS (st,st,st,st,o,o): fewer st<->o
    boundaries hide LDWEIGHTS turnarounds; st chunks alternate PE
    row-group replicas 0/32 so consecutive st LDWs overlap.
  - Epilogue per S-block: ACT copies the [65,512] fp32 accumulator to
    bf16 SBUF, PE transposes it back into the freed bank (66-wide bf16
    slots), DVE makes one 2x packed copy out, one reciprocal of the Z
    column and 4 scalar_tensor_tensor ops fuse 1/Z scaling with the
    residual add.
"""

import numpy as np

N = 4096
C = 64
D = 8
NCHUNK = 32  # m-chunks of 128
SBLK = 512  # n-block width
NS = N // SBLK  # 8 S-blocks
NCORES = 8
SPB = 16  # 2-chunk spans per S-block
NSPAN = NS * SPB  # 128

A8 = 11.54156457442115  # 8 * log2(e): e4m3 bytes per logit unit
B8C = 56.156  # e4m3 Schraudolph bias (incl. trunc + spline correction)
MARGIN = 4.5  # row max maps to e^4.5 ~ byte 108 (NaN zone starts at 120)

# fraction of exp spans on ScalarE (rest on VectorE via max->u8)
AFRAC = 0.585

_cache = {}


def _build_nc():
    import concourse.bacc as bacc
    import concourse.tile as tile
    from concourse import mybir

    f32 = mybir.dt.float32
    bf16 = mybir.dt.bfloat16
    fp8 = mybir.dt.float8e4
    u8 = mybir.dt.uint8
    EXP = mybir.ActivationFunctionType.Exp
    MUL = mybir.AluOpType.mult
    ADD = mybir.AluOpType.add
    MAX = mybir.AluOpType.max
    DR = mybir.MatmulPerfMode.DoubleRow

    nc = bacc.Bacc("TRN2", target_bir_lowering=False, debug=False)

    # xTb rows: 0-63 x^T, 64 ones (for biases + the f-side ones row),
    # 65 gshift (per-softmax-row Schraudolph shift, passed through the
    # g-projection).  wf9/wg9 [66, 9]: 8 projection columns + 1
    # passthrough column selecting the ones/gshift row.
    xr_ext = nc.declare_dram_parameter("xr", [128, NCHUNK, C], f32, isOutput=False)
    xTb_ext = nc.declare_dram_parameter("xTb", [C + 2, N], bf16, isOutput=False)
    wf9_ext = nc.declare_dram_parameter("wf9", [C + 2, D + 1], bf16, isOutput=False)
    wg9_ext = nc.declare_dram_parameter("wg9", [C + 2, D + 1], bf16, isOutput=False)
    whb_ext = nc.declare_dram_parameter("whb", [C + 2, C], bf16, isOutput=False)
    id_ext = nc.declare_dram_parameter("ident", [128, 128], bf16, isOutput=False)
    out_ext = nc.declare_dram_parameter("out", [N, C], f32, isOutput=True)

    # span -> engine assignment (True = ScalarE)
    span_on_act = []
    _acc = 0.0
    for _k in range(NSPAN):
        _acc += AFRAC
        if _acc >= 1.0:
            span_on_act.append(True)
            _acc -= 1.0
        else:
            span_on_act.append(False)

    with tile.TileContext(nc) as tc:
        with (
            tc.tile_pool(name="singles", bufs=1) as singles,
            tc.tile_pool(name="exp_sb", bufs=10) as exp_pool,
            tc.tile_pool(name="oT_sb", bufs=4) as oT_pool,
            tc.tile_pool(name="tr_sb", bufs=4) as tr_pool,
            tc.tile_pool(name="small", bufs=16) as small,
            tc.tile_pool(name="outsb", bufs=8) as out_pool,
        ):
            # ---- persistent SBUF tensors ----
            x_sb = singles.tile([128, NCHUNK, C], f32)
            wf9_sb = singles.tile([C + 2, D + 1], bf16)
            wg9_sb = singles.tile([C + 2, D + 1], bf16)
            xTb_sb = singles.tile([C + 2, N], bf16)
            whb_sb = singles.tile([C + 2, C], bf16)
            id_sb = singles.tile([128, 128], bf16)
            # f rows 0-7 + ones row 8; replica window at 32-40
            fT_sb = singles.tile([41, N], bf16)
            gT_sb = singles.tile([41, N], bf16)
            # h in fp8, 80-byte slots (pair stride %16==0 for DoubleRow
            # LDWEIGHTS); col 64 = Z-ones, cols 65-79 never read
            h_sb = singles.tile([128, NCHUNK, 80], fp8)
            dummy = singles.tile([128, 128], f32)

            # warm the ACT exp table while input DMAs run
            nc.vector.memset(dummy, 0.0)
            nc.scalar.activation(dummy[:, 0:1], dummy[:, 0:1], EXP)
            # per-partition bias AP for the ACT exp (fp32 -B8C/A8)
            ebias = singles.tile([128, 1], f32)
            nc.gpsimd.memset(ebias, -B8C / A8)

            # weights first on the fast HWDGE queues (sync) so the first
            # projection can start ASAP; bulk xT chunked right behind;
            # residual x on the scalar HWDGE queue; replica fan-outs go on
            # the gpsimd (SWDGE) queue
            nc.sync.dma_start(out=wf9_sb, in_=wf9_ext[:])
            nc.sync.dma_start(out=wg9_sb, in_=wg9_ext[:])
            nc.scalar.dma_start(out=whb_sb, in_=whb_ext[:])
            nc.scalar.dma_start(out=id_sb, in_=id_ext[:])
            for blk in range(NS):
                nc.sync.dma_start(
                    out=xTb_sb[:, blk * SBLK : (blk + 1) * SBLK],
                    in_=xTb_ext[:, blk * SBLK : (blk + 1) * SBLK],
                )
            # residual x rides last on sync (first needed ~30us in)
            nc.sync.dma_start(out=x_sb, in_=xr_ext[:])

            st_psum_cm = tc.tile_pool(name="st_psum", bufs=1, space="PSUM")
            st_psum = st_psum_cm.__enter__()
            # one tensor spanning all 8 PSUM banks; Tile tracks dependencies
            # at bank granularity.  banks 0-5: three rotating 2-bank st
            # slots; 6-7: o accumulators (and, during setup, f/g/h
            # production scratch)
            big = st_psum.tile([128, 8 * SBLK], f32)

            # warm the PE's HAM clock gate during the DMA ramp
            for _w in range(28):
                nc.tensor.matmul(
                    big[0:1, 7 * SBLK : 7 * SBLK + 128],
                    lhsT=dummy[:, 0:1],
                    rhs=dummy[:, 0:128],
                    start=True,
                    stop=True,
                )

            exp_tiles = [None] * NSPAN

            # ---- f / g projections: two [66,9] matmuls per block into one
            #      PSUM bank (partition bases 0 and 32 -> distinct PE
            #      col-groups, so they overlap), ACT copies to fT_sb/gT_sb
            #      rows 0-8 (f+ones / g+gshift), then SBUF->SBUF DMA
            #      fan-out to the row-32 replica windows ----
            for blk in range(NS):
                bcols = slice((6 + blk % 2) * SBLK, (6 + blk % 2) * SBLK + SBLK)
                psf = big[0 : D + 1, bcols]
                psg = big[32 : 32 + D + 1, bcols]
                nc.tensor.matmul(
                    psf,
                    lhsT=wf9_sb[:],
                    rhs=xTb_sb[:, blk * SBLK : (blk + 1) * SBLK],
                    start=True,
                    stop=True,
                )
                nc.tensor.matmul(
                    psg,
                    lhsT=wg9_sb[:],
                    rhs=xTb_sb[:, blk * SBLK : (blk + 1) * SBLK],
                    start=True,
                    stop=True,
                    tile_position=(0, 32),
                )
                q1 = slice(blk * SBLK, (blk + 1) * SBLK)
                nc.scalar.copy(fT_sb[0:9, q1], psf)
                nc.scalar.copy(gT_sb[0:9, q1], psg)
                if blk % 2 == 1:
                    q = slice((blk - 1) * SBLK, (blk + 1) * SBLK)
                    nc.gpsimd.dma_start(out=fT_sb[32:41, q], in_=fT_sb[0:9, q])
                    nc.scalar.dma_start(out=gT_sb[32:41, q], in_=gT_sb[0:9, q])

            # ---- h production: 4 chunk matmuls into bank-6/7 sub-slots,
            #      then one strided cast (fp32 -> fp8e4) moves all four.
            #      col 64 (Z-ones) via memset ----
            nc.gpsimd.memset(h_sb[:, :, C : C + 1], 1.0)

            def emit_h(t0):
                for t in range(t0, t0 + 4):
                    hps = big[:, 6 * SBLK + (t % 16) * C :
                              6 * SBLK + (t % 16 + 1) * C]
                    nc.tensor.matmul(
                        hps,
                        lhsT=xTb_sb[:, t * 128 : (t + 1) * 128],
                        rhs=whb_sb[:],
                        start=True,
                        stop=True,
                    )
                s0 = t0 % 16
                blk4 = big[:, 6 * SBLK + s0 * C : 6 * SBLK + (s0 + 4) * C]
                nc.vector.tensor_copy(
                    h_sb[:, t0 : t0 + 4, 0:C],
                    blk4.rearrange("p (b x) -> p b x", b=4),
                )

            for t0 in range(0, NCHUNK, 4):
                emit_h(t0)

            def emit_st(k):
                s, p = k // SPB, k % SPB
                base = (k % 3) * 2 * SBLK
                for j in range(2):
                    mc = 2 * p + j
                    nc.tensor.matmul(
                        big[:, base + j * SBLK : base + (j + 1) * SBLK],
                        lhsT=fT_sb[32 * j : 32 * j + D + 1,
                                   mc * 128 : (mc + 1) * 128],
                        rhs=gT_sb[32 * j : 32 * j + D + 1,
                                  s * SBLK : (s + 1) * SBLK],
                        start=True,
                        stop=True,
                        tile_position=(32 * j, 0),
                    )
                expt = exp_pool.tile([128, 2 * SBLK], fp8, tag="exp")
                exp_tiles[k] = expt
                if span_on_act[k]:
                    nc.scalar.activation(
                        expt[:], big[:, base : base + 2 * SBLK], EXP,
                        bias=ebias[:], scale=1.0 / A8,
                    )
                else:
                    nc.vector.tensor_scalar(
                        expt[:].bitcast(u8),
                        big[:, base : base + 2 * SBLK],
                        0.0,
                        None,
                        MAX,
                    )

            def emit_o(k):
                # one DoubleRow matmul accumulates the whole span
                s, p = k // SPB, k % SPB
                expt = exp_tiles[k]
                bank = 6 + (s % 2)
                acc = big[0 : C + 1, bank * SBLK : (bank + 1) * SBLK]
                nc.tensor.matmul(
                    acc,
                    lhsT=h_sb[:, 2 * p : 2 * p + 2, 0 : C + 1],
                    rhs=expt.rearrange("p (two n) -> p two n", two=2),
                    start=(p == 0),
                    stop=(p == SPB - 1),
                    perf_mode=DR,
                )

            def emit_epilogue(s):
                bank = 6 + (s % 2)
                acc = big[:, bank * SBLK : (bank + 1) * SBLK]
                # 1) ACT: accumulator -> bf16 SBUF
                oT = oT_pool.tile([C + 1, SBLK], bf16, tag="oT")
                nc.scalar.copy(oT[:], acc[0 : C + 1, :])
                # 2) PE: transpose back into the freed bank (bf16 view;
                #    66-wide slots keep PSUM accesses 4-byte aligned)
                trv = acc.bitcast(bf16)  # [128, 1024] bf16 view of the bank
                for j in range(4):
                    nc.tensor.transpose(
                        trv[:, j * 66 : j * 66 + (C + 1)],
                        in_=oT[:, j * 128 : (j + 1) * 128],
                        identity=id_sb[0 : C + 1, 0 : C + 1],
                    )
                # 3) DVE: one packed 2x copy out of PSUM
                tr = tr_pool.tile([128, 4, 66], bf16, tag="tr")
                nc.vector.tensor_copy(
                    tr[:],
                    trv[:, 0 : 4 * 66].rearrange("p (b x) -> p b x", b=4),
                )
                # 4) 1/Z and fused scale+residual
                rz4 = small.tile([128, 4, 1], f32, tag="rz")
                nc.vector.reciprocal(rz4, tr[:, :, C : C + 1])
                ot = out_pool.tile([128, 4, C], f32, tag="ot")
                for j in range(4):
                    nc.vector.scalar_tensor_tensor(
                        ot[:, j, :], tr[:, j, 0:C], rz4[:, j, :],
                        x_sb[:, s * 4 + j, :],
                        MUL, ADD,
                    )
                row = s * 512
                dview = out_ext[row : row + 512, :].rearrange(
                    "(b p) c -> p b c", p=128)
                nc.sync.dma_start(out=dview, in_=ot)

            # ---- main loop: st spans run 2 ahead of o-matmuls, emitted in
            #      groups (st,st,st,st,o,o) to limit st<->o PE boundaries;
            #      epilogues slot in one span after their S-block ends ----
            emit_st(0)
            emit_st(1)
            pending_epi = []
            for k in range(0, NSPAN, 2):
                for k2 in (k + 2, k + 3):
                    if k2 < NSPAN:
                        emit_st(k2)
                if pending_epi:
                    emit_epilogue(pending_epi.pop())
                for ko in (k, k + 1):
                    emit_o(ko)
                    if ko % SPB == SPB - 1:
                        pending_epi.append(ko // SPB)
            if pending_epi:
                emit_epilogue(pending_epi.pop())

            st_psum_cm.__exit__(None, None, None)

    nc.finalize()
    return nc


def _get_nc():
    if "nc" not in _cache:
        _cache["nc"] = _build_nc()
    return _cache["nc"]


def make_in_maps(x, kernel_f, kernel_g, kernel_h, bias_f, bias_g, bias_h, gamma):
    from ml_dtypes import bfloat16

    x = np.asarray(x, dtype=np.float32)
    gam = np.float32(np.asarray(gamma).reshape(-1)[0])
    c1 = np.float32(np.sqrt(A8))
    Wf = np.asarray(kernel_f, np.float32).reshape(C, D)
    Wg = np.asarray(kernel_g, np.float32).reshape(C, D)
    bf_ = np.asarray(bias_f, np.float32).reshape(D)
    bg_ = np.asarray(bias_g, np.float32).reshape(D)
    # [66, 9] projection weights: 8 scaled projection columns + a
    # passthrough column (col 8) selecting the ones row (f) / gshift row
    # (g) of xTb.  Row 65 multiplies the gshift row.
    wf9 = np.zeros((C + 2, D + 1), np.float32)
    wf9[0:C, 0:D] = Wf * c1
    wf9[C, 0:D] = bf_ * c1
    wf9[C, D] = 1.0  # ones row passthrough
    wg9 = np.zeros((C + 2, D + 1), np.float32)
    wg9[0:C, 0:D] = Wg * c1
    wg9[C, 0:D] = bg_ * c1
    wg9[C + 1, D] = 1.0  # gshift passthrough
    wh_aug = np.concatenate(
        [np.asarray(kernel_h, np.float32).reshape(C, C),
         np.asarray(bias_h, np.float32).reshape(1, C),
         np.zeros((1, C), np.float32)], axis=0) * gam
    whb = wh_aug  # [66, 64]; the Z-ones column of h comes from a memset
    ident = np.eye(128, dtype=np.float32)

    in_maps = []
    for i in range(NCORES):
        xf = x[i].reshape(N, C)
        xr = np.ascontiguousarray(xf.reshape(NCHUNK, 128, C).transpose(1, 0, 2))
        # exact softmax row maxes (cheap on host); the per-row shift rides
        # the st matmul as a 9th contraction row
        f = xf @ Wf + bf_
        g = xf @ Wg + bg_
        rowmax = (g @ f.T).max(axis=1)  # [N] over m
        gshift = -(A8 * (rowmax - MARGIN) - B8C)
        xT_aug = np.concatenate(
            [np.ascontiguousarray(xf.T), np.ones((1, N), np.float32),
             gshift.reshape(1, N).astype(np.float32)], axis=0)
        in_maps.append({
            "xr": xr, "xTb": xT_aug.astype(bfloat16),
            "wf9": wf9.astype(bfloat16), "wg9": wg9.astype(bfloat16),
            "whb": whb.astype(bfloat16),
            "ident": ident.astype(bfloat16),
        })
    return in_maps


def kernel(x, kernel_f, kernel_g, kernel_h, bias_f, bias_g, bias_h, gamma):
    from concourse.bass_utils import run_bass_kernel_spmd

    B, H, W, Cin = x.shape
    assert (B, H, W, Cin) == (8, 64, 64, 64)
    nc = _get_nc()
    in_maps = make_in_maps(x, kernel_f, kernel_g, kernel_h,
                           bias_f, bias_g, bias_h, gamma)
    res = run_bass_kernel_spmd(nc, in_maps, core_ids=list(range(NCORES)))
    out = np.stack([res.results[i]["out"] for i in range(NCORES)], axis=0)
    return out.reshape(B, H, W, Cin).astype(np.float32)
